# revision 1
# baseline (speedup 1.0000x reference)
"""ChannelAttentionBlock Trainium2 kernel.

Computes, per batch sample (x: [B=32, C=512, H=56, W=56] fp32, gamma: [1]):
    xh = max_w(x)                  # [C, H]
    xw = max_h(x)                  # [C, W]
    w1 = channel_attn(xh); w2 = channel_attn(xw)
    out = gamma * w1[:, :, None] * x * w2[:, None, :] + x
where channel_attn(f) = softmax(rowmax(aff) - aff, axis=-1) @ f, aff = f @ f.T.

Key algebra: softmax(rowmax - aff) == softmax(-aff) row-wise (shift invariant),
so with a global stabilizer K, e = exp(K - aff) is SYMMETRIC (aff is a Gram
matrix) and attn = e / rowsum(e). Symmetry lets the stored e tiles double as
the transposed lhsT for the second matmul (no 512x512 transposes). Row sums
come free from the ACT exp's accum_out. Normalization and gamma fold into
per-channel scales applied to the tiny [C, 56] pooled outputs.

Sharding: data-parallel over batch, 4 samples per core across 8 cores.

Engine split per core: DVE does both max-pool reduces, the outer-product
build, and the fused (t+1)*x combine; ACT does exp(+rowsum) and the small
PSUM->SBUF copies/scales; PE does the matmuls/transposes. (GpSimd tensor ops
and DMA-accumulate are rejected by this container's walrus build, so the
pools stay on DVE.)
"""

import numpy as np

import concourse.bass as bass
import concourse.tile as tile
from concourse import mybir
from concourse.masks import make_identity

f32 = mybir.dt.float32
P = 128
C = 512
H = 56
W = 56
CT = C // P          # 4 c-tiles
B_TOTAL = 32
N_CORES = 8
B_PER_CORE = B_TOTAL // N_CORES   # 4

K_STAB = 280.0       # global softmax stabilizer; safe window measured [232, 331]


def _build_sample(nc, tc, pools, b, x_in, out_dram, ident, gb, kb):
    sb, ps = pools["sb"], pools["ps"]
    Exp = mybir.ActivationFunctionType.Exp

    # ---- load the 4 c-tiles of x[b] -------------------------------------
    xts = []
    for i in range(CT):
        xt = sb.tile([P, H, W], f32, tag="x", bufs=8, name=f"x_{b}_{i}")
        nc.sync.dma_start(out=xt, in_=x_in[b, i * P : (i + 1) * P, :, :])
        xts.append(xt)

    # ---- pools: xh = max over w, xw = max over h (DVE reduces) ----------
    feat_h, feat_w = [], []
    for i in range(CT):
        fh = sb.tile([P, H], f32, tag="feat", bufs=16, name=f"fh_{b}_{i}")
        nc.vector.reduce_max(out=fh, in_=xts[i], axis=mybir.AxisListType.X)
        feat_h.append(fh)

        fw = sb.tile([P, W], f32, tag="feat", bufs=16, name=f"fw_{b}_{i}")
        nc.vector.reduce_max(
            out=fw, in_=xts[i].transpose([0, 2, 1]), axis=mybir.AxisListType.X
        )
        feat_w.append(fw)

    # ---- channel attention per branch -----------------------------------
    y_scaled = []  # per branch: scaled y in PSUM (h-branch) / SBUF (w-branch)
    rr_tiles = []
    es_all = []
    for br, feats in ((0, feat_h), (1, feat_w)):
        # featT [56, 512] via 4 PE transposes into one PSUM tile + 1 copy
        tpp = ps.tile([H, CT, P], f32, tag="mm", bufs=2, name=f"tp_{b}_{br}")
        for i in range(CT):
            nc.tensor.transpose(tpp[:, i, :], feats[i], ident)
        fT = sb.tile([H, C], f32, tag="fT", bufs=4, name=f"fT_{b}_{br}")
        nc.scalar.copy(out=fT, in_=tpp)

        # aff tiles + exp(K - aff) with row-sum accumulation
        rr = sb.tile([P, CT], f32, tag="rr", bufs=4, name=f"rr_{b}_{br}")
        es = []
        for i in range(CT):
            aff = ps.tile([P, C], f32, tag="mm", bufs=2, name=f"aff_{b}_{br}_{i}")
            nc.tensor.matmul(
                aff, lhsT=fT[:, i * P : (i + 1) * P], rhs=fT, start=True, stop=True
            )
            e = sb.tile([P, C], f32, tag="e", bufs=8, name=f"e_{b}_{br}_{i}")
            nc.scalar.activation(
                out=e, in_=aff, func=Exp, bias=kb, scale=-1.0,
                accum_out=rr[:, i : i + 1],
            )
            es.append(e)
        rr_tiles.append(rr)
        es_all.append(es)

        # y[:, i, :] = sum_j e^T-chunk @ feat  (e symmetric -> stored tiles)
        y_all = ps.tile([P, CT, W], f32, tag="y", bufs=2, name=f"y_{b}_{br}")
        for i in range(CT):
            for j in range(CT):
                nc.tensor.matmul(
                    y_all[:, i, :],
                    lhsT=es[j][:, i * P : (i + 1) * P],
                    rhs=feats[j],
                    start=(j == 0),
                    stop=(j == CT - 1),
                )
        y_scaled.append(y_all)

    # ---- per-channel scales ---------------------------------------------
    # s1 = gamma / r_h   (applied to y_h, in PSUM);  s2 = 1 / r_w (into SBUF)
    rec_h = sb.tile([P, CT], f32, tag="rec", bufs=4, name=f"rech_{b}")
    nc.vector.reciprocal(out=rec_h, in_=rr_tiles[0])
    s1 = sb.tile([P, CT], f32, tag="rec", bufs=4, name=f"s1_{b}")
    nc.vector.tensor_scalar_mul(out=s1, in0=rec_h, scalar1=gb)
    rec_w = sb.tile([P, CT], f32, tag="rec", bufs=4, name=f"recw_{b}")
    nc.vector.reciprocal(out=rec_w, in_=rr_tiles[1])

    # scale y tiles on ACT (keeps DVE free): y1q = y_h * s1, y2s = y_w * rec_w
    y1q = sb.tile([P, CT, H], f32, tag="y1q", bufs=4, name=f"y1q_{b}")
    for i in range(CT):
        nc.scalar.mul(out=y1q[:, i, :], in_=y_scaled[0][:, i, :], mul=s1[:, i : i + 1])
    y2s = sb.tile([P, CT, W], f32, tag="y2s", bufs=4, name=f"y2s_{b}")
    for i in range(CT):
        nc.scalar.mul(
            out=y2s[:, i, :], in_=y_scaled[1][:, i, :], mul=rec_w[:, i : i + 1]
        )

    # ---- combine: out = (t + 1) * x, t = y1q (x) y2s outer product ------
    for i in range(CT):
        ot = sb.tile([P, H, W], f32, tag="out", bufs=2, name=f"o_{b}_{i}")
        t = sb.tile([P, H, W], f32, tag="t", bufs=2, name=f"t_{b}_{i}")
        nc.vector.tensor_mul(
            out=t,
            in0=y2s[:, i, :].unsqueeze(1).broadcast_to((P, H, W)),
            in1=y1q[:, i, :].unsqueeze(2).broadcast_to((P, H, W)),
        )
        nc.vector.scalar_tensor_tensor(
            out=ot,
            in0=t,
            scalar=1.0,
            in1=xts[i],
            op0=mybir.AluOpType.add,
            op1=mybir.AluOpType.mult,
        )
        nc.sync.dma_start(out=out_dram[b, i * P : (i + 1) * P, :, :], in_=ot)


def _build():
    nc = bass.Bass()
    x_in = nc.dram_tensor("x", [B_PER_CORE, C, H, W], f32, kind="ExternalInput")
    g_in = nc.dram_tensor("gamma", [1], f32, kind="ExternalInput")
    out_dram = nc.dram_tensor(
        "out", [B_PER_CORE, C, H, W], f32, kind="ExternalOutput"
    )

    with tile.TileContext(nc) as tc:
        with (
            tc.tile_pool(name="consts", bufs=1) as consts,
            tc.tile_pool(name="sb", bufs=2) as sb,
            tc.tile_pool(name="ps", bufs=1, space="PSUM") as ps,
        ):
            ident = consts.tile([P, P], f32, tag="id", name="ident")
            make_identity(nc, ident)
            gb = consts.tile([P, 1], f32, tag="gb", name="gb")
            nc.sync.dma_start(out=gb, in_=g_in[:].to_broadcast((P, 1)))
            kb = consts.tile([P, 1], f32, tag="kb", name="kb")
            nc.vector.memset(kb, K_STAB)

            pools = {"sb": sb, "ps": ps}
            for b in range(B_PER_CORE):
                _build_sample(nc, tc, pools, b, x_in, out_dram, ident, gb, kb)
    return nc


def _split_attached_waits(raw: bytes) -> bytes:
    """Move every attached on_wait into a standalone EventSemaphore instruction
    placed directly before its owner (same engine stream, same semantics: the
    sequencer blocks, then dispatches the op). The walrus build in this
    environment rejects instructions whose EVENTS struct carries more sync-wait
    commands than it has slots; standalone one-wait EventSemaphore instructions
    are the raw-bass style it always accepts."""
    import json

    bir = json.loads(raw)
    for fn in bir["functions"]:
        for blk in fn["blocks"]:
            new = []
            for inst in blk["instructions"]:
                si = inst.get("sync_info")
                ow = (si or {}).get("on_wait") or []
                if ow and inst.get("opcode") != "EventSemaphore":
                    for k, w in enumerate(ow):
                        new.append(
                            {
                                "debug": inst.get("debug", 0),
                                "engine": inst["engine"],
                                "ins": [],
                                "outs": [],
                                "name": f"{inst['name']}_sw{k}",
                                "opcode": "EventSemaphore",
                                "sync_info": {"on_update": [], "on_wait": [w]},
                            }
                        )
                    si["on_wait"] = []
                new.append(inst)
            blk["instructions"] = new
    return json.dumps(bir).encode()


_NC_CACHE = None


def _get_nc():
    global _NC_CACHE
    if _NC_CACHE is None:
        nc = _build()
        orig = nc.to_json_bytes
        nc.to_json_bytes = lambda: _split_attached_waits(orig())
        _NC_CACHE = nc
    return _NC_CACHE


def kernel(x, gamma):
    from concourse.bass_utils import run_bass_kernel_spmd

    x = np.ascontiguousarray(np.asarray(x), dtype=np.float32)
    gamma = np.ascontiguousarray(np.asarray(gamma), dtype=np.float32)
    nc = _get_nc()
    in_maps = [
        {"x": x[c * B_PER_CORE : (c + 1) * B_PER_CORE], "gamma": gamma}
        for c in range(N_CORES)
    ]
    res = run_bass_kernel_spmd(nc, in_maps, core_ids=list(range(N_CORES)))
    return np.concatenate([r["out"] for r in res.results], axis=0)



# revision 6
# speedup vs baseline: 1.5142x; 1.5142x over previous
"""ChannelAttentionBlock Trainium2 kernel (fp16 pipeline, DVE/ACT balanced).

Computes, per batch sample (x: [B=32, C=512, H=56, W=56] fp32, gamma: [1]):
    xh = max_w(x); xw = max_h(x)               # [C, H], [C, W]
    w1 = channel_attn(xh); w2 = channel_attn(xw)
    out = gamma * w1[:, :, None] * x * w2[:, None, :] + x
where channel_attn(f) = softmax(rowmax(aff) - aff, axis=-1) @ f, aff = f @ f.T.

Numerics: softmax(rowmax - aff) == softmax(-aff) row-wise, so with a global
stabilizer K, e = exp(K - aff) is SYMMETRIC and attn = e / rowsum(e); the
stored e tiles double as the transposed lhsT for the second matmul. Row sums
come free from the ACT exp's accum_out.

Precision (rel-err gate 2e-2; measured ~4e-3 end to end):
- x converts to fp16 on the host: halves DMA both ways AND makes every DVE
  TensorTensor eligible for the 2x "2x_1p" perf mode (2-byte packed operands).
  fp16 max-pools are exact picks; the fp16 rounding of x feeds the aff Gram
  matrix whose softmax needs ~1e-3 feature precision (bf16 feats fail the
  gate at 2.1e-2, fp16 passes at ~4e-3).
- e tiles are bf16 (values reach ~4e15, overflowing fp16); y-matmul rhs feats
  are bf16 copies. aff/rowsum/PSUM stay fp32.

Engine split per core (4 samples, software-pipelined with lookahead 2 so the
last sample's attention latency overlaps earlier combines):
- DVE: max-pools as fp16 TensorTensor-max trees batched over all 4 c-tiles
  (56->28->14->7 + 7-wide TensorReduce tail) at 2x; combine u = x*y2s,
  v-rows [0:DVE_ROWS) = u*y1d, o = x + v at 2x. y1 is stored as duplicated
  pairs y1d[p,h,j]=y1[p,h] so its W-broadcast AP keeps a packed innermost dim.
- ACT: exp(+rowsum accum), fT copies, scale folds, and v-rows [DVE_ROWS:56)
  as per-row per-partition-scale Copy ops (balances DVE vs ACT occupancy).
- PE: fp16 transposes and matmuls at 1 cycle/row.
GpSimd tensor ops and DMA-accumulate are rejected by this container's walrus
build, so nothing runs on Pool.

Sharding: data-parallel over batch, 4 samples per core across 8 cores.
"""

import numpy as np

import concourse.bass as bass
import concourse.tile as tile
from concourse import mybir
from concourse.masks import make_identity

f32 = mybir.dt.float32
f16 = mybir.dt.float16
bf16 = mybir.dt.bfloat16
Alu = mybir.AluOpType
P = 128
C = 512
H = 56
W = 56
CT = C // P          # 4 c-tiles
B_TOTAL = 32
N_CORES = 8
B_PER_CORE = B_TOTAL // N_CORES   # 4

K_STAB = 280.0       # global softmax stabilizer; safe window measured [232, 331]
DVE_ROWS = 56        # combine v-pass rows on DVE (56 = all; ACT offload regressed: latency coupling)


def _load_and_trees(nc, sb, b, x_in, st):
    """DMA sample b's x and build both pooled features (batched over c-tiles)."""
    xall = sb.tile([P, CT, H, W], f16, tag="x", bufs=3, name=f"x_{b}")
    for i in range(CT):
        nc.sync.dma_start(out=xall[:, i, :, :], in_=x_in[b, i * P : (i + 1) * P, :, :])

    # Tree level 1 runs per c-tile so it can start as soon as that c-tile's
    # DMA lands; deeper levels are batched over all 4 c-tiles per op.
    m1 = sb.tile([P, CT, H, 28], f16, tag="m1", bufs=1, name=f"m1_{b}")
    n1 = sb.tile([P, CT, 28, W], f16, tag="n1", bufs=1, name=f"n1_{b}")
    for i in range(CT):
        nc.vector.tensor_tensor(
            out=m1[:, i, :, :], in0=xall[:, i, :, 0:28],
            in1=xall[:, i, :, 28:56], op=Alu.max)
        nc.vector.tensor_tensor(
            out=n1[:, i, :, :], in0=xall[:, i, 0:28, :],
            in1=xall[:, i, 28:56, :], op=Alu.max)

    # W-direction tree (reduce innermost W)
    m2 = sb.tile([P, CT, H, 14], f16, tag="m2", bufs=1, name=f"m2_{b}")
    nc.vector.tensor_tensor(
        out=m2, in0=m1[:, :, :, 0:14], in1=m1[:, :, :, 14:28], op=Alu.max)
    m3 = sb.tile([P, CT, H, 7], f16, tag="m3", bufs=1, name=f"m3_{b}")
    nc.vector.tensor_tensor(
        out=m3, in0=m2[:, :, :, 0:7], in1=m2[:, :, :, 7:14], op=Alu.max)
    feat_h = sb.tile([P, CT, H], f16, tag="feat", bufs=8, name=f"fh_{b}")
    nc.vector.reduce_max(out=feat_h, in_=m3, axis=mybir.AxisListType.X)

    # H-direction tree (reduce middle H)
    n2 = sb.tile([P, CT, 14, W], f16, tag="n2", bufs=1, name=f"n2_{b}")
    nc.vector.tensor_tensor(
        out=n2, in0=n1[:, :, 0:14, :], in1=n1[:, :, 14:28, :], op=Alu.max)
    n3 = sb.tile([P, CT, 7, W], f16, tag="n3", bufs=1, name=f"n3_{b}")
    nc.vector.tensor_tensor(
        out=n3, in0=n2[:, :, 0:7, :], in1=n2[:, :, 7:14, :], op=Alu.max)
    feat_w = sb.tile([P, CT, W], f16, tag="feat", bufs=8, name=f"fw_{b}")
    nc.vector.reduce_max(
        out=feat_w, in_=n3.transpose([0, 1, 3, 2]), axis=mybir.AxisListType.X)

    # bf16 copies for the y-matmul rhs (4x DVE copy)
    featb_h = sb.tile([P, CT, H], bf16, tag="featb", bufs=8, name=f"fhb_{b}")
    nc.vector.tensor_copy(out=featb_h, in_=feat_h)
    featb_w = sb.tile([P, CT, W], bf16, tag="featb", bufs=8, name=f"fwb_{b}")
    nc.vector.tensor_copy(out=featb_w, in_=feat_w)

    st[b] = (xall, feat_h, feat_w, featb_h, featb_w)


def _attn_and_combine(nc, sb, ps, b, out_dram, ident16, gb, kb, st):
    xall, feat_h, feat_w, featb_h, featb_w = st.pop(b)
    Exp = mybir.ActivationFunctionType.Exp
    Copy = mybir.ActivationFunctionType.Copy

    # ---- channel attention per branch -----------------------------------
    y_psum = []
    rr_tiles = []
    for br, feats, featb in ((0, feat_h, featb_h), (1, feat_w, featb_w)):
        tpp = ps.tile([H, CT, P], f16, tag="tp", bufs=2, name=f"tp_{b}_{br}")
        for i in range(CT):
            nc.tensor.transpose(tpp[:, i, :], feats[:, i, :], ident16)
        fT = sb.tile([H, C], f16, tag="fT", bufs=4, name=f"fT_{b}_{br}")
        nc.scalar.copy(out=fT, in_=tpp)

        rr = sb.tile([P, CT], f32, tag="rr", bufs=4, name=f"rr_{b}_{br}")
        es = []
        for i in range(CT):
            aff = ps.tile([P, C], f32, tag="mm", bufs=2, name=f"aff_{b}_{br}_{i}")
            nc.tensor.matmul(
                aff, lhsT=fT[:, i * P : (i + 1) * P], rhs=fT, start=True, stop=True)
            e = sb.tile([P, C], bf16, tag="e", bufs=8, name=f"e_{b}_{br}_{i}")
            nc.scalar.activation(
                out=e, in_=aff, func=Exp, bias=kb, scale=-1.0,
                accum_out=rr[:, i : i + 1])
            es.append(e)
        rr_tiles.append(rr)

        y_all = ps.tile([P, CT, W], f32, tag="y", bufs=2, name=f"y_{b}_{br}")
        for i in range(CT):
            for j in range(CT):
                nc.tensor.matmul(
                    y_all[:, i, :],
                    lhsT=es[j][:, i * P : (i + 1) * P],
                    rhs=featb[:, j, :],
                    start=(j == 0), stop=(j == CT - 1))
        y_psum.append(y_all)

    # ---- per-channel scales ---------------------------------------------
    rec_h = sb.tile([P, CT], f32, tag="rec", bufs=4, name=f"rech_{b}")
    nc.vector.reciprocal(out=rec_h, in_=rr_tiles[0])
    s1 = sb.tile([P, CT], f32, tag="rec", bufs=4, name=f"s1_{b}")
    nc.vector.tensor_scalar_mul(out=s1, in0=rec_h, scalar1=gb)
    rec_w = sb.tile([P, CT], f32, tag="rec", bufs=4, name=f"recw_{b}")
    nc.vector.reciprocal(out=rec_w, in_=rr_tiles[1])

    # ACT folds scales while casting to fp16. y1d holds duplicated pairs for
    # the DVE v-rows; y1f holds plain fp32 per-(c,h) scales for the ACT rows.
    y1d = sb.tile([P, CT, H, 2], f16, tag="y1d", bufs=4, name=f"y1d_{b}")
    y2s = sb.tile([P, CT, W], f16, tag="y2s", bufs=4, name=f"y2s_{b}")
    for i in range(CT):
        nc.scalar.activation(
            out=y1d[:, i, :, :],
            in_=y_psum[0][:, i, :].unsqueeze(2).broadcast_to((P, H, 2)),
            func=Copy, scale=s1[:, i : i + 1])
        nc.scalar.activation(
            out=y2s[:, i, :], in_=y_psum[1][:, i, :],
            func=Copy, scale=rec_w[:, i : i + 1])

    # ---- combine: out = x + (x * y2s_bcast) * y1_bcast (fp16 2x) --------
    for i in range(CT):
        u = sb.tile([P, H, W], f16, tag="u", bufs=2, name=f"u_{b}_{i}")
        nc.vector.tensor_tensor(
            out=u, in0=xall[:, i, :, :],
            in1=y2s[:, i, :].unsqueeze(1).broadcast_to((P, H, W)), op=Alu.mult)
        v = sb.tile([P, H, W], f16, tag="v", bufs=2, name=f"v_{b}_{i}")
        nc.vector.tensor_tensor(
            out=v, in0=u,
            in1=y1d[:, i, :, :].unsqueeze(2).broadcast_to((P, H, 28, 2)),
            op=Alu.mult)
        o = sb.tile([P, H, W], f16, tag="o", bufs=2, name=f"o_{b}_{i}")
        nc.vector.tensor_tensor(out=o, in0=xall[:, i, :, :], in1=v, op=Alu.add)
        nc.sync.dma_start(out=out_dram[b, i * P : (i + 1) * P, :, :], in_=o)


def _build():
    nc = bass.Bass()
    x_in = nc.dram_tensor("x", [B_PER_CORE, C, H, W], f16, kind="ExternalInput")
    g_in = nc.dram_tensor("gamma", [1], f32, kind="ExternalInput")
    out_dram = nc.dram_tensor(
        "out", [B_PER_CORE, C, H, W], f16, kind="ExternalOutput")

    with tile.TileContext(nc) as tc:
        with (
            tc.tile_pool(name="consts", bufs=1) as consts,
            tc.tile_pool(name="sb", bufs=2) as sb,
            tc.tile_pool(name="ps", bufs=1, space="PSUM") as ps,
        ):
            ident16 = consts.tile([P, P], f16, tag="id16", name="ident16")
            make_identity(nc, ident16)
            gb = consts.tile([P, 1], f32, tag="gb", name="gb")
            nc.sync.dma_start(out=gb, in_=g_in[:].to_broadcast((P, 1)))
            kb = consts.tile([P, 1], f32, tag="kb", name="kb")
            nc.vector.memset(kb, K_STAB)

            # software pipeline: trees run 2 samples ahead of attention so the
            # final sample's attention latency hides under earlier combines
            st = {}
            _load_and_trees(nc, sb, 0, x_in, st)
            if B_PER_CORE > 1:
                _load_and_trees(nc, sb, 1, x_in, st)
            for b in range(B_PER_CORE):
                _attn_and_combine(nc, sb, ps, b, out_dram, ident16, gb, kb, st)
                if b + 2 < B_PER_CORE:
                    _load_and_trees(nc, sb, b + 2, x_in, st)
    return nc


def _split_attached_waits(raw: bytes) -> bytes:
    """Move every attached on_wait into a standalone EventSemaphore instruction
    placed directly before its owner (same engine stream, same semantics: the
    sequencer blocks, then dispatches the op). The walrus build in this
    environment rejects instructions whose EVENTS struct carries more sync-wait
    commands than it has slots; standalone one-wait EventSemaphore instructions
    are the raw-bass style it always accepts."""
    import json

    bir = json.loads(raw)
    for fn in bir["functions"]:
        for blk in fn["blocks"]:
            new = []
            for inst in blk["instructions"]:
                si = inst.get("sync_info")
                ow = (si or {}).get("on_wait") or []
                if ow and inst.get("opcode") != "EventSemaphore":
                    for k, w in enumerate(ow):
                        new.append(
                            {
                                "debug": inst.get("debug", 0),
                                "engine": inst["engine"],
                                "ins": [],
                                "outs": [],
                                "name": f"{inst['name']}_sw{k}",
                                "opcode": "EventSemaphore",
                                "sync_info": {"on_update": [], "on_wait": [w]},
                            }
                        )
                    si["on_wait"] = []
                new.append(inst)
            blk["instructions"] = new
    return json.dumps(bir).encode()


_NC_CACHE = None


def _get_nc():
    global _NC_CACHE
    if _NC_CACHE is None:
        nc = _build()
        orig = nc.to_json_bytes
        nc.to_json_bytes = lambda: _split_attached_waits(orig())
        _NC_CACHE = nc
    return _NC_CACHE


def kernel(x, gamma):
    from concourse.bass_utils import run_bass_kernel_spmd

    x = np.asarray(x, dtype=np.float32).astype(np.float16)
    gamma = np.ascontiguousarray(np.asarray(gamma), dtype=np.float32)
    nc = _get_nc()
    in_maps = [
        {"x": x[c * B_PER_CORE : (c + 1) * B_PER_CORE], "gamma": gamma}
        for c in range(N_CORES)
    ]
    res = run_bass_kernel_spmd(nc, in_maps, core_ids=list(range(N_CORES)))
    return np.concatenate(
        [np.asarray(r["out"]).astype(np.float32) for r in res.results], axis=0
    )


# revision 11
# speedup vs baseline: 1.5595x; 1.0299x over previous
"""ChannelAttentionBlock Trainium2 kernel (fp16 pipeline, DVE/ACT balanced).

Computes, per batch sample (x: [B=32, C=512, H=56, W=56] fp32, gamma: [1]):
    xh = max_w(x); xw = max_h(x)               # [C, H], [C, W]
    w1 = channel_attn(xh); w2 = channel_attn(xw)
    out = gamma * w1[:, :, None] * x * w2[:, None, :] + x
where channel_attn(f) = softmax(rowmax(aff) - aff, axis=-1) @ f, aff = f @ f.T.

Numerics: softmax(rowmax - aff) == softmax(-aff) row-wise, so with a global
stabilizer K, e = exp(K - aff) is SYMMETRIC and attn = e / rowsum(e); the
stored e tiles double as the transposed lhsT for the second matmul. Row sums
come free from the ACT exp's accum_out.

Precision (rel-err gate 2e-2; measured ~4e-3 end to end):
- x converts to fp16 on the host: halves DMA both ways AND makes every DVE
  TensorTensor eligible for the 2x "2x_1p" perf mode (2-byte packed operands).
  fp16 max-pools are exact picks; the fp16 rounding of x feeds the aff Gram
  matrix whose softmax needs ~1e-3 feature precision (bf16 feats fail the
  gate at 2.1e-2, fp16 passes at ~4e-3).
- e tiles are bf16 (values reach ~4e15, overflowing fp16); y-matmul rhs feats
  are bf16 copies. aff/rowsum/PSUM stay fp32.

Engine split per core (4 samples, software-pipelined with lookahead 2 so the
last sample's attention latency overlaps earlier combines):
- DVE: max-pools as fp16 TensorTensor-max trees batched over all 4 c-tiles
  (56->28->14->7 + 7-wide TensorReduce tail) at 2x; combine u = x*y2s,
  v-rows [0:DVE_ROWS) = u*y1d, o = x + v at 2x. y1 is stored as duplicated
  pairs y1d[p,h,j]=y1[p,h] so its W-broadcast AP keeps a packed innermost dim.
- ACT: exp(+rowsum accum), fT copies, scale folds, and v-rows [DVE_ROWS:56)
  as per-row per-partition-scale Copy ops (balances DVE vs ACT occupancy).
- PE: fp16 transposes and matmuls at 1 cycle/row.
GpSimd tensor ops and DMA-accumulate are rejected by this container's walrus
build, so nothing runs on Pool.

Sharding: data-parallel over batch, 4 samples per core across 8 cores.
"""

import numpy as np

import concourse.bass as bass
import concourse.tile as tile
from concourse import mybir
from concourse.masks import make_identity

f32 = mybir.dt.float32
f16 = mybir.dt.float16
bf16 = mybir.dt.bfloat16
Alu = mybir.AluOpType
P = 128
C = 512
H = 56
W = 56
CT = C // P          # 4 c-tiles
B_TOTAL = 32
N_CORES = 8
B_PER_CORE = B_TOTAL // N_CORES   # 4

K_STAB = 280.0       # global softmax stabilizer; safe window measured [232, 331]
DVE_ROWS = 56        # combine v-pass rows on DVE (56 = all; ACT offload regressed: latency coupling)


def _load_and_trees(nc, sb, b, x_in, st):
    """DMA sample b's x and build both pooled features (batched over c-tiles)."""
    xall = sb.tile([P, CT, H, W], f16, tag="x", bufs=3, name=f"x_{b}")
    for i in range(CT):
        if b == 0 and i == 0:
            # halve the pipeline-fill stall: first c-tile lands in two DMAs
            nc.sync.dma_start(
                out=xall[:, 0, 0:28, :], in_=x_in[0, 0:P, 0:28, :])
            nc.sync.dma_start(
                out=xall[:, 0, 28:56, :], in_=x_in[0, 0:P, 28:56, :])
        else:
            nc.sync.dma_start(
                out=xall[:, i, :, :], in_=x_in[b, i * P : (i + 1) * P, :, :])

    # Tree level 1: per c-tile for sample 0 (starts right after each c-tile's
    # DMA, hiding the fill); fully batched once the pipeline is running.
    m1 = sb.tile([P, CT, H, 28], f16, tag="m1", bufs=1, name=f"m1_{b}")
    n1 = sb.tile([P, CT, 28, W], f16, tag="n1", bufs=1, name=f"n1_{b}")
    if b == 0:
        for i in range(CT):
            if i == 0:
                # m1 in h-halves, each gated on one half-DMA of c-tile 0
                nc.vector.tensor_tensor(
                    out=m1[:, 0, 0:28, :], in0=xall[:, 0, 0:28, 0:28],
                    in1=xall[:, 0, 0:28, 28:56], op=Alu.max)
                nc.vector.tensor_tensor(
                    out=m1[:, 0, 28:56, :], in0=xall[:, 0, 28:56, 0:28],
                    in1=xall[:, 0, 28:56, 28:56], op=Alu.max)
            else:
                nc.vector.tensor_tensor(
                    out=m1[:, i, :, :], in0=xall[:, i, :, 0:28],
                    in1=xall[:, i, :, 28:56], op=Alu.max)
            nc.vector.tensor_tensor(
                out=n1[:, i, :, :], in0=xall[:, i, 0:28, :],
                in1=xall[:, i, 28:56, :], op=Alu.max)
    else:
        nc.vector.tensor_tensor(
            out=m1, in0=xall[:, :, :, 0:28], in1=xall[:, :, :, 28:56], op=Alu.max)
        nc.vector.tensor_tensor(
            out=n1, in0=xall[:, :, 0:28, :], in1=xall[:, :, 28:56, :], op=Alu.max)

    # W-direction tree (reduce innermost W); the 7-wide tail folds via w-slice
    # max ops (first one 2x-packed, last three tiny 1x single-column ops),
    # cheaper than a 7-wide TensorReduce at 1x.
    m2 = sb.tile([P, CT, H, 14], f16, tag="m2", bufs=1, name=f"m2_{b}")
    nc.vector.tensor_tensor(
        out=m2, in0=m1[:, :, :, 0:14], in1=m1[:, :, :, 14:28], op=Alu.max)
    m3 = sb.tile([P, CT, H, 7], f16, tag="m3", bufs=1, name=f"m3_{b}")
    nc.vector.tensor_tensor(
        out=m3, in0=m2[:, :, :, 0:7], in1=m2[:, :, :, 7:14], op=Alu.max)
    mr3 = sb.tile([P, CT, H, 3], f16, tag="mr3", bufs=1, name=f"mr3_{b}")
    nc.vector.tensor_tensor(
        out=mr3, in0=m3[:, :, :, 0:3], in1=m3[:, :, :, 3:6], op=Alu.max)
    mf1 = sb.tile([P, CT, H], f16, tag="mf", bufs=2, name=f"mf1_{b}")
    nc.vector.tensor_tensor(
        out=mf1, in0=mr3[:, :, :, 0], in1=mr3[:, :, :, 1], op=Alu.max)
    mf2 = sb.tile([P, CT, H], f16, tag="mf", bufs=2, name=f"mf2_{b}")
    nc.vector.tensor_tensor(
        out=mf2, in0=mf1, in1=mr3[:, :, :, 2], op=Alu.max)
    feat_h = sb.tile([P, CT, H], f16, tag="feat", bufs=8, name=f"fh_{b}")
    nc.vector.tensor_tensor(
        out=feat_h, in0=mf2, in1=m3[:, :, :, 6], op=Alu.max)

    # H-direction tree (reduce middle H); tail slices stay w-innermost packed
    # so every tail op keeps the 2x mode.
    n2 = sb.tile([P, CT, 14, W], f16, tag="n2", bufs=1, name=f"n2_{b}")
    nc.vector.tensor_tensor(
        out=n2, in0=n1[:, :, 0:14, :], in1=n1[:, :, 14:28, :], op=Alu.max)
    n3 = sb.tile([P, CT, 7, W], f16, tag="n3", bufs=1, name=f"n3_{b}")
    nc.vector.tensor_tensor(
        out=n3, in0=n2[:, :, 0:7, :], in1=n2[:, :, 7:14, :], op=Alu.max)
    nr3 = sb.tile([P, CT, 3, W], f16, tag="nr3", bufs=1, name=f"nr3_{b}")
    nc.vector.tensor_tensor(
        out=nr3, in0=n3[:, :, 0:3, :], in1=n3[:, :, 4:7, :], op=Alu.max)
    nf1 = sb.tile([P, CT, W], f16, tag="nf", bufs=2, name=f"nf1_{b}")
    nc.vector.tensor_tensor(
        out=nf1, in0=nr3[:, :, 0, :], in1=nr3[:, :, 1, :], op=Alu.max)
    nf2 = sb.tile([P, CT, W], f16, tag="nf", bufs=2, name=f"nf2_{b}")
    nc.vector.tensor_tensor(
        out=nf2, in0=nf1, in1=nr3[:, :, 2, :], op=Alu.max)
    feat_w = sb.tile([P, CT, W], f16, tag="feat", bufs=8, name=f"fw_{b}")
    nc.vector.tensor_tensor(
        out=feat_w, in0=nf2, in1=n3[:, :, 3, :], op=Alu.max)

    # bf16 copies for the y-matmul rhs (on ACT; keeps DVE clear)
    featb_h = sb.tile([P, CT, H], bf16, tag="featb", bufs=8, name=f"fhb_{b}")
    nc.scalar.copy(out=featb_h, in_=feat_h)
    featb_w = sb.tile([P, CT, W], bf16, tag="featb", bufs=8, name=f"fwb_{b}")
    nc.scalar.copy(out=featb_w, in_=feat_w)

    st[b] = (xall, feat_h, feat_w, featb_h, featb_w)


def _attn_and_combine(nc, sb, ps, b, out_dram, ident16, gb, kb, st):
    xall, feat_h, feat_w, featb_h, featb_w = st.pop(b)
    Exp = mybir.ActivationFunctionType.Exp
    Copy = mybir.ActivationFunctionType.Copy

    # ---- channel attention per branch -----------------------------------
    y_psum = []
    rr_tiles = []
    for br, feats, featb in ((0, feat_h, featb_h), (1, feat_w, featb_w)):
        tpp = ps.tile([H, CT, P], f16, tag="tp", bufs=2, name=f"tp_{b}_{br}")
        for i in range(CT):
            nc.tensor.transpose(tpp[:, i, :], feats[:, i, :], ident16)
        fT = sb.tile([H, C], f16, tag="fT", bufs=4, name=f"fT_{b}_{br}")
        nc.scalar.copy(out=fT, in_=tpp)

        rr = sb.tile([P, CT], f32, tag="rr", bufs=4, name=f"rr_{b}_{br}")
        es = []
        for i in range(CT):
            aff = ps.tile([P, C], f32, tag="mm", bufs=2, name=f"aff_{b}_{br}_{i}")
            nc.tensor.matmul(
                aff, lhsT=fT[:, i * P : (i + 1) * P], rhs=fT, start=True, stop=True)
            e = sb.tile([P, C], bf16, tag="e", bufs=8, name=f"e_{b}_{br}_{i}")
            nc.scalar.activation(
                out=e, in_=aff, func=Exp, bias=kb, scale=-1.0,
                accum_out=rr[:, i : i + 1])
            es.append(e)
        rr_tiles.append(rr)

        y_all = ps.tile([P, CT, W], f32, tag="y", bufs=2, name=f"y_{b}_{br}")
        for i in range(CT):
            for j in range(CT):
                nc.tensor.matmul(
                    y_all[:, i, :],
                    lhsT=es[j][:, i * P : (i + 1) * P],
                    rhs=featb[:, j, :],
                    start=(j == 0), stop=(j == CT - 1))
        y_psum.append(y_all)

    # ---- per-channel scales ---------------------------------------------
    rec_h = sb.tile([P, CT], f32, tag="rec", bufs=4, name=f"rech_{b}")
    nc.vector.reciprocal(out=rec_h, in_=rr_tiles[0])
    s1 = sb.tile([P, CT], f32, tag="rec", bufs=4, name=f"s1_{b}")
    nc.scalar.mul(out=s1, in_=rec_h, mul=gb)
    rec_w = sb.tile([P, CT], f32, tag="rec", bufs=4, name=f"recw_{b}")
    nc.vector.reciprocal(out=rec_w, in_=rr_tiles[1])

    # ACT folds scales while casting to fp16. y1d holds duplicated pairs for
    # the DVE v-rows; y1f holds plain fp32 per-(c,h) scales for the ACT rows.
    y1d = sb.tile([P, CT, H, 2], f16, tag="y1d", bufs=4, name=f"y1d_{b}")
    y2s = sb.tile([P, CT, W], f16, tag="y2s", bufs=4, name=f"y2s_{b}")
    for i in range(CT):
        nc.scalar.activation(
            out=y1d[:, i, :, :],
            in_=y_psum[0][:, i, :].unsqueeze(2).broadcast_to((P, H, 2)),
            func=Copy, scale=s1[:, i : i + 1])
        nc.scalar.activation(
            out=y2s[:, i, :], in_=y_psum[1][:, i, :],
            func=Copy, scale=rec_w[:, i : i + 1])

    # ---- combine: out = x + (x * y2s_bcast) * y1_bcast (fp16 2x) --------
    for i in range(CT):
        u = sb.tile([P, H, W], f16, tag="u", bufs=2, name=f"u_{b}_{i}")
        nc.vector.tensor_tensor(
            out=u, in0=xall[:, i, :, :],
            in1=y2s[:, i, :].unsqueeze(1).broadcast_to((P, H, W)), op=Alu.mult)
        v = sb.tile([P, H, W], f16, tag="v", bufs=2, name=f"v_{b}_{i}")
        nc.vector.tensor_tensor(
            out=v, in0=u,
            in1=y1d[:, i, :, :].unsqueeze(2).broadcast_to((P, H, 28, 2)),
            op=Alu.mult)
        o = sb.tile([P, H, W], f16, tag="o", bufs=2, name=f"o_{b}_{i}")
        if b == B_PER_CORE - 1 and i == CT - 1:
            # final c-tile: compute+ship in h-halves to shorten the drain tail
            nc.vector.tensor_tensor(
                out=o[:, 0:28, :], in0=xall[:, i, 0:28, :], in1=v[:, 0:28, :],
                op=Alu.add)
            nc.sync.dma_start(
                out=out_dram[b, i * P : (i + 1) * P, 0:28, :], in_=o[:, 0:28, :])
            nc.vector.tensor_tensor(
                out=o[:, 28:56, :], in0=xall[:, i, 28:56, :], in1=v[:, 28:56, :],
                op=Alu.add)
            nc.sync.dma_start(
                out=out_dram[b, i * P : (i + 1) * P, 28:56, :], in_=o[:, 28:56, :])
        else:
            nc.vector.tensor_tensor(out=o, in0=xall[:, i, :, :], in1=v, op=Alu.add)
            nc.sync.dma_start(out=out_dram[b, i * P : (i + 1) * P, :, :], in_=o)


def _build():
    nc = bass.Bass()
    x_in = nc.dram_tensor("x", [B_PER_CORE, C, H, W], f16, kind="ExternalInput")
    g_in = nc.dram_tensor("gamma", [1], f32, kind="ExternalInput")
    out_dram = nc.dram_tensor(
        "out", [B_PER_CORE, C, H, W], f16, kind="ExternalOutput")

    with tile.TileContext(nc) as tc:
        with (
            tc.tile_pool(name="consts", bufs=1) as consts,
            tc.tile_pool(name="sb", bufs=2) as sb,
            tc.tile_pool(name="ps", bufs=1, space="PSUM") as ps,
        ):
            ident16 = consts.tile([P, P], f16, tag="id16", name="ident16")
            make_identity(nc, ident16)
            gb = consts.tile([P, 1], f32, tag="gb", name="gb")
            nc.sync.dma_start(out=gb, in_=g_in[:].to_broadcast((P, 1)))
            kb = consts.tile([P, 1], f32, tag="kb", name="kb")
            nc.vector.memset(kb, K_STAB)

            # software pipeline: trees run 2 samples ahead of attention so the
            # final sample's attention latency hides under earlier combines
            st = {}
            _load_and_trees(nc, sb, 0, x_in, st)
            if B_PER_CORE > 1:
                _load_and_trees(nc, sb, 1, x_in, st)
            for b in range(B_PER_CORE):
                _attn_and_combine(nc, sb, ps, b, out_dram, ident16, gb, kb, st)
                if b + 2 < B_PER_CORE:
                    _load_and_trees(nc, sb, b + 2, x_in, st)
    return nc


def _split_attached_waits(raw: bytes) -> bytes:
    """Move every attached on_wait into a standalone EventSemaphore instruction
    placed directly before its owner (same engine stream, same semantics: the
    sequencer blocks, then dispatches the op). The walrus build in this
    environment rejects instructions whose EVENTS struct carries more sync-wait
    commands than it has slots; standalone one-wait EventSemaphore instructions
    are the raw-bass style it always accepts."""
    import json

    bir = json.loads(raw)
    for fn in bir["functions"]:
        for blk in fn["blocks"]:
            new = []
            for inst in blk["instructions"]:
                si = inst.get("sync_info")
                ow = (si or {}).get("on_wait") or []
                if ow and inst.get("opcode") != "EventSemaphore":
                    for k, w in enumerate(ow):
                        new.append(
                            {
                                "debug": inst.get("debug", 0),
                                "engine": inst["engine"],
                                "ins": [],
                                "outs": [],
                                "name": f"{inst['name']}_sw{k}",
                                "opcode": "EventSemaphore",
                                "sync_info": {"on_update": [], "on_wait": [w]},
                            }
                        )
                    si["on_wait"] = []
                new.append(inst)
            blk["instructions"] = new
    return json.dumps(bir).encode()


_NC_CACHE = None


def _get_nc():
    global _NC_CACHE
    if _NC_CACHE is None:
        nc = _build()
        orig = nc.to_json_bytes
        nc.to_json_bytes = lambda: _split_attached_waits(orig())
        _NC_CACHE = nc
    return _NC_CACHE


def kernel(x, gamma):
    from concourse.bass_utils import run_bass_kernel_spmd

    x = np.asarray(x, dtype=np.float32).astype(np.float16)
    gamma = np.ascontiguousarray(np.asarray(gamma), dtype=np.float32)
    nc = _get_nc()
    in_maps = [
        {"x": x[c * B_PER_CORE : (c + 1) * B_PER_CORE], "gamma": gamma}
        for c in range(N_CORES)
    ]
    res = run_bass_kernel_spmd(nc, in_maps, core_ids=list(range(N_CORES)))
    return np.concatenate(
        [np.asarray(r["out"]).astype(np.float32) for r in res.results], axis=0
    )


# revision 17
# speedup vs baseline: 1.5779x; 1.0118x over previous
"""ChannelAttentionBlock Trainium2 kernel (fp16 pipeline, DVE/ACT balanced).

Computes, per batch sample (x: [B=32, C=512, H=56, W=56] fp32, gamma: [1]):
    xh = max_w(x); xw = max_h(x)               # [C, H], [C, W]
    w1 = channel_attn(xh); w2 = channel_attn(xw)
    out = gamma * w1[:, :, None] * x * w2[:, None, :] + x
where channel_attn(f) = softmax(rowmax(aff) - aff, axis=-1) @ f, aff = f @ f.T.

Numerics: softmax(rowmax - aff) == softmax(-aff) row-wise, so with a global
stabilizer K, e = exp(K - aff) is SYMMETRIC and attn = e / rowsum(e); the
stored e tiles double as the transposed lhsT for the second matmul. Row sums
come free from the ACT exp's accum_out.

Precision (rel-err gate 2e-2; measured ~4e-3 end to end):
- x converts to fp16 on the host: halves DMA both ways AND makes every DVE
  TensorTensor eligible for the 2x "2x_1p" perf mode (2-byte packed operands).
  fp16 max-pools are exact picks; the fp16 rounding of x feeds the aff Gram
  matrix whose softmax needs ~1e-3 feature precision (bf16 feats fail the
  gate at 2.1e-2, fp16 passes at ~4e-3).
- e tiles are bf16 (values reach ~4e15, overflowing fp16); y-matmul rhs feats
  are bf16 copies. aff/rowsum/PSUM stay fp32.

Engine split per core (4 samples, software-pipelined with lookahead 2 so the
last sample's attention latency overlaps earlier combines):
- DVE: max-pools as fp16 TensorTensor-max trees batched over all 4 c-tiles
  (56->28->14->7 + 7-wide TensorReduce tail) at 2x; combine u = x*y2s,
  v-rows [0:DVE_ROWS) = u*y1d, o = x + v at 2x. y1 is stored as duplicated
  pairs y1d[p,h,j]=y1[p,h] so its W-broadcast AP keeps a packed innermost dim.
- ACT: exp(+rowsum accum), fT copies, scale folds, and v-rows [DVE_ROWS:56)
  as per-row per-partition-scale Copy ops (balances DVE vs ACT occupancy).
- PE: fp16 transposes and matmuls at 1 cycle/row.
GpSimd tensor ops and DMA-accumulate are rejected by this container's walrus
build, so nothing runs on Pool.

Sharding: data-parallel over batch, 4 samples per core across 8 cores.
"""

import numpy as np

import concourse.bass as bass
import concourse.tile as tile
from concourse import mybir
from concourse.masks import make_identity

f32 = mybir.dt.float32
f16 = mybir.dt.float16
bf16 = mybir.dt.bfloat16
Alu = mybir.AluOpType
P = 128
C = 512
H = 56
W = 56
CT = C // P          # 4 c-tiles
B_TOTAL = 32
N_CORES = 8
B_PER_CORE = B_TOTAL // N_CORES   # 4

K_STAB = 280.0       # global softmax stabilizer; safe window measured [232, 331]
ACT_V_CTILE = -1     # c-tile whose combine v-pass runs on ACT (-1: none; ACT in-order queue makes any offload delay the exp chain)


def _load_and_trees(nc, sb, b, x_in, st):
    """DMA sample b's x and build both pooled features (batched over c-tiles)."""
    xall = sb.tile([P, CT, H, W], f16, tag="x", bufs=3, name=f"x_{b}")
    for i in range(CT):
        if b == 0 and i == 0:
            # halve the pipeline-fill stall: first c-tile lands in two DMAs
            nc.sync.dma_start(
                out=xall[:, 0, 0:28, :], in_=x_in[0, 0:P, 0:28, :])
            nc.sync.dma_start(
                out=xall[:, 0, 28:56, :], in_=x_in[0, 0:P, 28:56, :])
        else:
            nc.sync.dma_start(
                out=xall[:, i, :, :], in_=x_in[b, i * P : (i + 1) * P, :, :])

    # Tree level 1: per c-tile for sample 0 (starts right after each c-tile's
    # DMA, hiding the fill); fully batched once the pipeline is running.
    m1 = sb.tile([P, CT, H, 28], f16, tag="m1", bufs=1, name=f"m1_{b}")
    n1 = sb.tile([P, CT, 28, W], f16, tag="n1", bufs=1, name=f"n1_{b}")
    if b == 0:
        for i in range(CT):
            if i == 0:
                # m1 in h-halves, each gated on one half-DMA of c-tile 0
                nc.vector.tensor_tensor(
                    out=m1[:, 0, 0:28, :], in0=xall[:, 0, 0:28, 0:28],
                    in1=xall[:, 0, 0:28, 28:56], op=Alu.max)
                nc.vector.tensor_tensor(
                    out=m1[:, 0, 28:56, :], in0=xall[:, 0, 28:56, 0:28],
                    in1=xall[:, 0, 28:56, 28:56], op=Alu.max)
            else:
                nc.vector.tensor_tensor(
                    out=m1[:, i, :, :], in0=xall[:, i, :, 0:28],
                    in1=xall[:, i, :, 28:56], op=Alu.max)
            nc.vector.tensor_tensor(
                out=n1[:, i, :, :], in0=xall[:, i, 0:28, :],
                in1=xall[:, i, 28:56, :], op=Alu.max)
    else:
        nc.vector.tensor_tensor(
            out=m1, in0=xall[:, :, :, 0:28], in1=xall[:, :, :, 28:56], op=Alu.max)
        nc.vector.tensor_tensor(
            out=n1, in0=xall[:, :, 0:28, :], in1=xall[:, :, 28:56, :], op=Alu.max)

    # W-direction tree (reduce innermost W); the 7-wide tail folds via w-slice
    # max ops (first one 2x-packed, last three tiny 1x single-column ops),
    # cheaper than a 7-wide TensorReduce at 1x.
    m2 = sb.tile([P, CT, H, 14], f16, tag="m2", bufs=1, name=f"m2_{b}")
    nc.vector.tensor_tensor(
        out=m2, in0=m1[:, :, :, 0:14], in1=m1[:, :, :, 14:28], op=Alu.max)
    m3 = sb.tile([P, CT, H, 7], f16, tag="m3", bufs=1, name=f"m3_{b}")
    nc.vector.tensor_tensor(
        out=m3, in0=m2[:, :, :, 0:7], in1=m2[:, :, :, 7:14], op=Alu.max)
    mr3 = sb.tile([P, CT, H, 3], f16, tag="mr3", bufs=1, name=f"mr3_{b}")
    nc.vector.tensor_tensor(
        out=mr3, in0=m3[:, :, :, 0:3], in1=m3[:, :, :, 3:6], op=Alu.max)
    mf1 = sb.tile([P, CT, H], f16, tag="mf", bufs=2, name=f"mf1_{b}")
    nc.vector.tensor_tensor(
        out=mf1, in0=mr3[:, :, :, 0], in1=mr3[:, :, :, 1], op=Alu.max)
    mf2 = sb.tile([P, CT, H], f16, tag="mf", bufs=2, name=f"mf2_{b}")
    nc.vector.tensor_tensor(
        out=mf2, in0=mf1, in1=mr3[:, :, :, 2], op=Alu.max)
    feat_h = sb.tile([P, CT, H], f16, tag="feat", bufs=8, name=f"fh_{b}")
    nc.vector.tensor_tensor(
        out=feat_h, in0=mf2, in1=m3[:, :, :, 6], op=Alu.max)

    # H-direction tree (reduce middle H); tail slices stay w-innermost packed
    # so every tail op keeps the 2x mode.
    n2 = sb.tile([P, CT, 14, W], f16, tag="n2", bufs=1, name=f"n2_{b}")
    nc.vector.tensor_tensor(
        out=n2, in0=n1[:, :, 0:14, :], in1=n1[:, :, 14:28, :], op=Alu.max)
    n3 = sb.tile([P, CT, 7, W], f16, tag="n3", bufs=1, name=f"n3_{b}")
    nc.vector.tensor_tensor(
        out=n3, in0=n2[:, :, 0:7, :], in1=n2[:, :, 7:14, :], op=Alu.max)
    nr3 = sb.tile([P, CT, 3, W], f16, tag="nr3", bufs=1, name=f"nr3_{b}")
    nc.vector.tensor_tensor(
        out=nr3, in0=n3[:, :, 0:3, :], in1=n3[:, :, 4:7, :], op=Alu.max)
    nf1 = sb.tile([P, CT, W], f16, tag="nf", bufs=2, name=f"nf1_{b}")
    nc.vector.tensor_tensor(
        out=nf1, in0=nr3[:, :, 0, :], in1=nr3[:, :, 1, :], op=Alu.max)
    nf2 = sb.tile([P, CT, W], f16, tag="nf", bufs=2, name=f"nf2_{b}")
    nc.vector.tensor_tensor(
        out=nf2, in0=nf1, in1=nr3[:, :, 2, :], op=Alu.max)
    feat_w = sb.tile([P, CT, W], f16, tag="feat", bufs=8, name=f"fw_{b}")
    nc.vector.tensor_tensor(
        out=feat_w, in0=nf2, in1=n3[:, :, 3, :], op=Alu.max)

    # bf16 copies for the y-matmul rhs (on ACT; keeps DVE clear)
    featb_h = sb.tile([P, CT, H], bf16, tag="featb", bufs=8, name=f"fhb_{b}")
    nc.scalar.copy(out=featb_h, in_=feat_h)
    featb_w = sb.tile([P, CT, W], bf16, tag="featb", bufs=8, name=f"fwb_{b}")
    nc.scalar.copy(out=featb_w, in_=feat_w)

    st[b] = (xall, feat_h, feat_w, featb_h, featb_w)


def _attn_and_combine(nc, sb, ps, b, out_dram, ident16, gb, kb, st):
    xall, feat_h, feat_w, featb_h, featb_w = st.pop(b)
    Exp = mybir.ActivationFunctionType.Exp
    Copy = mybir.ActivationFunctionType.Copy

    # ---- channel attention per branch -----------------------------------
    # W-branch first: the combine's first op u = x*y2s needs only y2s, so
    # emitting the W chain early lets DVE start u during the H attention.
    y_psum = {}
    rr_tiles = {}
    for br, feats, featb in ((1, feat_w, featb_w), (0, feat_h, featb_h)):
        tpp = ps.tile([H, CT, P], f16, tag="tp", bufs=2, name=f"tp_{b}_{br}")
        for i in range(CT):
            nc.tensor.transpose(tpp[:, i, :], feats[:, i, :], ident16)
        fT = sb.tile([H, C], f16, tag="fT", bufs=4, name=f"fT_{b}_{br}")
        nc.scalar.copy(out=fT, in_=tpp)

        rr = sb.tile([P, CT], f32, tag="rr", bufs=4, name=f"rr_{b}_{br}")
        es = []
        for i in range(CT):
            aff = ps.tile([P, C], f32, tag="mm", bufs=2, name=f"aff_{b}_{br}_{i}")
            nc.tensor.matmul(
                aff, lhsT=fT[:, i * P : (i + 1) * P], rhs=fT, start=True, stop=True)
            e = sb.tile([P, C], bf16, tag="e", bufs=8, name=f"e_{b}_{br}_{i}")
            nc.scalar.activation(
                out=e, in_=aff, func=Exp, bias=kb, scale=-1.0,
                accum_out=rr[:, i : i + 1])
            es.append(e)
        rr_tiles[br] = rr

        y_all = ps.tile([P, CT, W], f32, tag="y", bufs=2, name=f"y_{b}_{br}")
        for i in range(CT):
            for j in range(CT):
                nc.tensor.matmul(
                    y_all[:, i, :],
                    lhsT=es[j][:, i * P : (i + 1) * P],
                    rhs=featb[:, j, :],
                    start=(j == 0), stop=(j == CT - 1))
        y_psum.append(y_all)

    # ---- per-channel scales ---------------------------------------------
    rec_h = sb.tile([P, CT], f32, tag="rec", bufs=4, name=f"rech_{b}")
    nc.vector.reciprocal(out=rec_h, in_=rr_tiles[0])
    s1 = sb.tile([P, CT], f32, tag="rec", bufs=4, name=f"s1_{b}")
    nc.scalar.mul(out=s1, in_=rec_h, mul=gb)
    rec_w = sb.tile([P, CT], f32, tag="rec", bufs=4, name=f"recw_{b}")
    nc.vector.reciprocal(out=rec_w, in_=rr_tiles[1])

    # ACT folds scales while casting to fp16. y1d holds duplicated pairs for
    # the DVE v-rows; y1f holds plain fp32 per-(c,h) scales for the ACT rows.
    y1d = sb.tile([P, CT, H, 2], f16, tag="y1d", bufs=4, name=f"y1d_{b}")
    y1f = sb.tile([P, H], f32, tag="y1f", bufs=4, name=f"y1f_{b}")
    y2s = sb.tile([P, CT, W], f16, tag="y2s", bufs=4, name=f"y2s_{b}")
    for i in range(CT):
        if i == ACT_V_CTILE:
            nc.scalar.activation(
                out=y1f, in_=y_psum[0][:, i, :],
                func=Copy, scale=s1[:, i : i + 1])
        else:
            nc.scalar.activation(
                out=y1d[:, i, :, :],
                in_=y_psum[0][:, i, :].unsqueeze(2).broadcast_to((P, H, 2)),
                func=Copy, scale=s1[:, i : i + 1])
        nc.scalar.activation(
            out=y2s[:, i, :], in_=y_psum[1][:, i, :],
            func=Copy, scale=rec_w[:, i : i + 1])

    # ---- combine: out = x + (x * y2s_bcast) * y1_bcast (fp16 2x) --------
    for i in range(CT):
        u = sb.tile([P, H, W], f16, tag="u", bufs=2, name=f"u_{b}_{i}")
        nc.vector.tensor_tensor(
            out=u, in0=xall[:, i, :, :],
            in1=y2s[:, i, :].unsqueeze(1).broadcast_to((P, H, W)), op=Alu.mult)
        v = sb.tile([P, H, W], f16, tag="v", bufs=2, name=f"v_{b}_{i}")
        if i == ACT_V_CTILE:
            # this c-tile's v runs as 56 per-row scale ops on ACT, freeing DVE;
            # DVE fills with the other c-tiles and the next sample's trees
            for h in range(H):
                nc.scalar.activation(
                    out=v[:, h, :], in_=u[:, h, :], func=Copy,
                    scale=y1f[:, h : h + 1])
        else:
            nc.vector.tensor_tensor(
                out=v, in0=u,
                in1=y1d[:, i, :, :].unsqueeze(2).broadcast_to((P, H, 28, 2)),
                op=Alu.mult)
        o = sb.tile([P, H, W], f16, tag="o", bufs=2, name=f"o_{b}_{i}")
        if b == B_PER_CORE - 1 and i == CT - 1:
            # final c-tile: compute+ship in h-halves to shorten the drain tail
            nc.vector.tensor_tensor(
                out=o[:, 0:28, :], in0=xall[:, i, 0:28, :], in1=v[:, 0:28, :],
                op=Alu.add)
            nc.sync.dma_start(
                out=out_dram[b, i * P : (i + 1) * P, 0:28, :], in_=o[:, 0:28, :])
            nc.vector.tensor_tensor(
                out=o[:, 28:56, :], in0=xall[:, i, 28:56, :], in1=v[:, 28:56, :],
                op=Alu.add)
            nc.sync.dma_start(
                out=out_dram[b, i * P : (i + 1) * P, 28:56, :], in_=o[:, 28:56, :])
        else:
            nc.vector.tensor_tensor(out=o, in0=xall[:, i, :, :], in1=v, op=Alu.add)
            nc.sync.dma_start(out=out_dram[b, i * P : (i + 1) * P, :, :], in_=o)


def _build():
    nc = bass.Bass()
    x_in = nc.dram_tensor("x", [B_PER_CORE, C, H, W], f16, kind="ExternalInput")
    g_in = nc.dram_tensor("gamma", [1], f32, kind="ExternalInput")
    out_dram = nc.dram_tensor(
        "out", [B_PER_CORE, C, H, W], f16, kind="ExternalOutput")

    with tile.TileContext(nc) as tc:
        with (
            tc.tile_pool(name="consts", bufs=1) as consts,
            tc.tile_pool(name="sb", bufs=2) as sb,
            tc.tile_pool(name="ps", bufs=1, space="PSUM") as ps,
        ):
            ident16 = consts.tile([P, P], f16, tag="id16", name="ident16")
            make_identity(nc, ident16)
            gb = consts.tile([P, 1], f32, tag="gb", name="gb")
            nc.sync.dma_start(out=gb, in_=g_in[:].to_broadcast((P, 1)))
            kb = consts.tile([P, 1], f32, tag="kb", name="kb")
            nc.vector.memset(kb, K_STAB)

            # software pipeline: trees run 2 samples ahead of attention so the
            # final sample's attention latency hides under earlier combines
            st = {}
            _load_and_trees(nc, sb, 0, x_in, st)
            if B_PER_CORE > 1:
                _load_and_trees(nc, sb, 1, x_in, st)
            for b in range(B_PER_CORE):
                _attn_and_combine(nc, sb, ps, b, out_dram, ident16, gb, kb, st)
                if b + 2 < B_PER_CORE:
                    _load_and_trees(nc, sb, b + 2, x_in, st)
    return nc


def _split_attached_waits(raw: bytes) -> bytes:
    """Move every attached on_wait into a standalone EventSemaphore instruction
    placed directly before its owner (same engine stream, same semantics: the
    sequencer blocks, then dispatches the op). The walrus build in this
    environment rejects instructions whose EVENTS struct carries more sync-wait
    commands than it has slots; standalone one-wait EventSemaphore instructions
    are the raw-bass style it always accepts."""
    import json

    bir = json.loads(raw)
    for fn in bir["functions"]:
        for blk in fn["blocks"]:
            new = []
            for inst in blk["instructions"]:
                si = inst.get("sync_info")
                ow = (si or {}).get("on_wait") or []
                if ow and inst.get("opcode") != "EventSemaphore":
                    for k, w in enumerate(ow):
                        new.append(
                            {
                                "debug": inst.get("debug", 0),
                                "engine": inst["engine"],
                                "ins": [],
                                "outs": [],
                                "name": f"{inst['name']}_sw{k}",
                                "opcode": "EventSemaphore",
                                "sync_info": {"on_update": [], "on_wait": [w]},
                            }
                        )
                    si["on_wait"] = []
                new.append(inst)
            blk["instructions"] = new
    return json.dumps(bir).encode()


_NC_CACHE = None


def _get_nc():
    global _NC_CACHE
    if _NC_CACHE is None:
        nc = _build()
        orig = nc.to_json_bytes
        nc.to_json_bytes = lambda: _split_attached_waits(orig())
        _NC_CACHE = nc
    return _NC_CACHE


def kernel(x, gamma):
    from concourse.bass_utils import run_bass_kernel_spmd

    x = np.asarray(x, dtype=np.float32).astype(np.float16)
    gamma = np.ascontiguousarray(np.asarray(gamma), dtype=np.float32)
    nc = _get_nc()
    in_maps = [
        {"x": x[c * B_PER_CORE : (c + 1) * B_PER_CORE], "gamma": gamma}
        for c in range(N_CORES)
    ]
    res = run_bass_kernel_spmd(nc, in_maps, core_ids=list(range(N_CORES)))
    return np.concatenate(
        [np.asarray(r["out"]).astype(np.float32) for r in res.results], axis=0
    )
